# revision 16
# baseline (speedup 1.0000x reference)
"""MultiHeadAttention (dense, B=4 S=2048 D=1024 H=16) + residual + LayerNorm
on 8 Trainium2 NeuronCores.

Sharding: core c handles batch b=c//2 and head group g=c%2 (8 of 16 heads),
all 2048 query tokens.  fc partials (K=512) are pair-exchanged with a
ReduceScatter so each core finishes residual+LayerNorm for its own 1024
tokens.

Perf design (v2 — PE-density rewrite):
  - The PE HAM clock gate drops the array to 1.2 GHz after any ~3.4us idle
    window and only returns to 2.4 GHz after a fully-busy window.  The
    v1 kernel stalled the PE on exp->att@V dependencies and on epilogue
    work that blocked the in-order vector queue, so it ran cold (K=4/8)
    for 85% of the kernel.  v2 keeps the PE stream gap-free:
      * att@V trails the score matmuls by 2 key-chunks (software pipeline)
        so EXPT values are always ready when the PE reaches them.
      * epilogue work (normalize / fc / ReduceScatter / LayerNorm) is
        emitted 1-3 head-pair windows after it becomes runnable, so no
        in-order engine queue ever parks on a collective or DMA.
  - matmul dtype choice follows the measured row rates: bf16 (FWL hidden
    weight loads) for the dk=64 score matmuls; fp8e4 DoubleRow only where
    the contraction is 256 (projections, att@V, fc).
  - softmax exp splits across vector (Schraudolph uint8 bit-trick),
    scalar (exact Exp) and gpsimd (bit-trick) ~ 44/44/12.
  - PSUM: sc 2x[128,512] + pv 2x[65,1024] + psf 1x[128,1024] = 8 banks.
  - att@V lhsT carries a ones-column (=2.0) so pv row 64 accumulates
    2*sum(exp): softmax denominators come free; reciprocal runs per
    head-pair and hides under the next head-pair's attention.
"""

import numpy as np
import ml_dtypes

import concourse.bass as bass
import concourse.mybir as mybir
import concourse.tile as tile
from concourse import bacc
from concourse.bass_utils import run_bass_kernel_spmd

BF16 = mybir.dt.bfloat16
F32 = mybir.dt.float32
FP8 = mybir.dt.float8e4
FP8E5 = mybir.dt.float8e5
U8 = mybir.dt.uint8
AF = mybir.ActivationFunctionType
OP = mybir.AluOpType
DR = mybir.MatmulPerfMode.DoubleRow

B = 4
S = 2048
D = 1024
HL = 8          # heads per core
DK = 64
DH = HL * DK    # 512 local projection width
P = 128
KC = S // P     # 16 key chunks
QC = 2          # query halves of 1024
LN_EPS = 1e-5
LN2 = float(np.log(2.0))
A_QK = float(np.sqrt(4.0 / (8.0 * LN2)))   # per-side Q/K scale
C2 = 59.8                                   # exp trick: t = max(s' + C2, 0)
WS = 32.0                                   # host weight scale for fp8
EXP_SCALE = LN2 / 4.0                       # scalar-engine exp scale

_NC_CACHE = None
_LAST_RES = None


def build_nc():
    nc = bacc.Bacc(
        None, target_bir_lowering=False, num_devices=8, dynamic_dma_scratch_size=2048
    )

    xqT = nc.declare_dram_parameter("xqT", [D, S], FP8, isOutput=False)
    xkT = nc.declare_dram_parameter("xkT", [D, S], FP8, isOutput=False)
    xvT = nc.declare_dram_parameter("xvT", [D, S], FP8, isOutput=False)
    wqT = nc.declare_dram_parameter("wqT", [D, DH], FP8, isOutput=False)
    wkT = nc.declare_dram_parameter("wkT", [D, DH], FP8, isOutput=False)
    wvT = nc.declare_dram_parameter("wvT", [D, DH], FP8, isOutput=False)
    woT = nc.declare_dram_parameter("woT", [DH, D], FP8, isOutput=False)
    bq_d = nc.declare_dram_parameter("bq", [DH], F32, isOutput=False)   # *A_QK
    bk_d = nc.declare_dram_parameter("bk", [DH], F32, isOutput=False)   # *A_QK
    bv_d = nc.declare_dram_parameter("bv", [1, DH], F32, isOutput=False)  # *32
    gam_d = nc.declare_dram_parameter("gamma", [1, D], F32, isOutput=False)
    bet_d = nc.declare_dram_parameter("beta", [1, D], F32, isOutput=False)
    qrb_d = nc.declare_dram_parameter("qrb", [S // 2, D], BF16, isOutput=False)
    out_d = nc.declare_dram_parameter("out", [S // 2, D], F32, isOutput=True)

    sums_dram = nc.dram_tensor("sums_dram", [HL * QC, 1024], F32)
    rec_dram = nc.dram_tensor("rec_dram", [HL * QC, 1024], F32)
    # ReduceScatter bounce buffers, 2 chunks per q-half (bf16 partials).
    # chunk rows = [256 tokens of pair-member 0, 256 tokens of member 1] so the
    # scatter hands each core its own tokens.
    cc_in = [nc.dram_tensor(f"cc_in{i}", [512, D], BF16) for i in range(2 * QC)]
    cc_out = [nc.dram_tensor(f"cc_out{i}", [256, D], BF16) for i in range(2 * QC)]
    groups = [[0, 1], [2, 3], [4, 5], [6, 7]]

    # softmax-exp engine assignment per (kc, hb, half) unit.
    # gpsimd cannot read PSUM, so exp is vector/scalar only: 7/16 vs 9/16
    # (scalar's activation is slightly faster and its other load is lighter).
    def exp_engine(idx):
        if idx % 2 == 0:
            return "vec"
        return "sca"

    with tile.TileContext(nc) as tc:
        with tc.tile_pool(name="pers", bufs=1) as pers:
            QT = pers.tile([P, 4, S], BF16, tag="QT")
            KT = pers.tile([P, 4, S], BF16, tag="KT")
            # V fp8 x32, DoubleRow pair layout [p, kpair, ktile, head, 66]
            VO = pers.tile([P, 8, 2, HL, 66], FP8, tag="VO")
            # normalized context (x16), fp8, d_local = chunk*128 + p
            ATT = pers.tile([P, 4, S], FP8, tag="ATT")
            WOp = pers.tile([P, 4, D], FP8, tag="WOp")
            BQK = pers.tile([P, 8], F32, tag="BQK")  # cols 0-3 bq*a, 4-7 bk*a

            nc.gpsimd.memset(VO[:, :, :, :, 64:65], 2.0)

            # ---------------- projections (fp8 DoubleRow) ----------------
            with (
                tc.tile_pool(name="inp", bufs=1) as inp,
                tc.tile_pool(name="pj", bufs=2, space="PSUM") as pj,
            ):
                XQ = inp.tile([P, 8, S], FP8, tag="XQ")
                XK = inp.tile([P, 8, S], FP8, tag="XK")
                XV = inp.tile([P, 8, S], FP8, tag="XV")
                WQ = inp.tile([P, 8, DH], FP8, tag="WQ")
                WK = inp.tile([P, 8, DH], FP8, tag="WK")
                WV = inp.tile([P, 8, DH], FP8, tag="WV")
                BVB = inp.tile([P, DH], F32, tag="BVB")

                nc.sync.dma_start(out=WV, in_=wvT.ap().rearrange("(c p) n -> p c n", p=P))
                nc.sync.dma_start(out=BVB, in_=bv_d.ap().to_broadcast([P, DH]))
                # chunk XV by 256-token slices so V-proj starts early
                for tq in range(8):
                    nc.sync.dma_start(
                        out=XV[:, :, tq * 256 : (tq + 1) * 256],
                        in_=xvT.ap().rearrange("(c p) s -> p c s", p=P)[
                            :, :, tq * 256 : (tq + 1) * 256
                        ],
                    )
                nc.sync.dma_start(out=WQ, in_=wqT.ap().rearrange("(c p) n -> p c n", p=P))
                nc.sync.dma_start(out=WK, in_=wkT.ap().rearrange("(c p) n -> p c n", p=P))
                nc.sync.dma_start(out=BQK[:, 0:4], in_=bq_d.ap().rearrange("(c p) -> p c", p=P))
                nc.sync.dma_start(out=BQK[:, 4:8], in_=bk_d.ap().rearrange("(c p) -> p c", p=P))
                for tq in range(2):
                    nc.sync.dma_start(
                        out=XQ[:, :, tq * 1024 : (tq + 1) * 1024],
                        in_=xqT.ap().rearrange("(c p) s -> p c s", p=P)[
                            :, :, tq * 1024 : (tq + 1) * 1024
                        ],
                    )
                for tq in range(2):
                    nc.sync.dma_start(
                        out=XK[:, :, tq * 1024 : (tq + 1) * 1024],
                        in_=xkT.ap().rearrange("(c p) s -> p c s", p=P)[
                            :, :, tq * 1024 : (tq + 1) * 1024
                        ],
                    )
                nc.sync.dma_start(out=WOp, in_=woT.ap().rearrange("(c p) d -> p c d", p=P))

                # V = (v @ Wv.T)*32 + 32*bv, sliced per head into VO (fp8)
                for tokc in range(KC):
                    psv = pj.tile([P, DH], F32, tag="psv", name="psv")
                    for kp in range(4):
                        nc.tensor.matmul(
                            psv,
                            lhsT=XV[:, 2 * kp : 2 * kp + 2, tokc * P : (tokc + 1) * P],
                            rhs=WV[:, 2 * kp : 2 * kp + 2, :],
                            start=(kp == 0),
                            stop=(kp == 3),
                            perf_mode=DR,
                        )
                    nc.vector.tensor_tensor(
                        VO[:, tokc // 2, tokc % 2, :, 0:DK],
                        psv.rearrange("p (h d) -> p h d", h=HL),
                        BVB.rearrange("p (h d) -> p h d", h=HL),
                        OP.add,
                    )

                # Q^T / K^T = a*(W @ x^T + b)  (d_out on partitions), bf16
                for mc in range(4):
                    for which, WX, XX, outT, bcol in (
                        (0, WQ, XQ, QT, 0),
                        (1, WK, XK, KT, 4),
                    ):
                        for nt in range(2):
                            psq = pj.tile([P, 1024], F32, tag="psq", name="psq")
                            for kp in range(4):
                                for half in range(2):
                                    nc.tensor.matmul(
                                        psq[:, half * 512 : (half + 1) * 512],
                                        lhsT=WX[:, 2 * kp : 2 * kp + 2, mc * P : (mc + 1) * P],
                                        rhs=XX[
                                            :,
                                            2 * kp : 2 * kp + 2,
                                            nt * 1024 + half * 512 : nt * 1024 + (half + 1) * 512,
                                        ],
                                        start=(kp == 0),
                                        stop=(kp == 3),
                                        perf_mode=DR,
                                    )
                            nc.vector.tensor_scalar(
                                out=outT[:, mc, nt * 1024 : (nt + 1) * 1024],
                                in0=psq,
                                scalar1=A_QK / WS,
                                scalar2=BQK[:, bcol + mc : bcol + mc + 1],
                                op0=OP.mult,
                                op1=OP.add,
                            )

            # ---------------- attention + exchange + fc ----------------
            with (
                tc.tile_pool(name="attp", bufs=1) as attp,
                tc.tile_pool(name="late", bufs=1) as late,
                tc.tile_pool(name="ps", bufs=2, space="PSUM") as ps,
            ):
                GAM = late.tile([P, D], F32, tag="GAM")
                BET = late.tile([P, D], F32, tag="BET")
                QRB = late.tile([P, 8, D], BF16, tag="QRB")
                nc.sync.dma_start(out=GAM, in_=gam_d.ap().to_broadcast([P, D]))
                nc.sync.dma_start(out=BET, in_=bet_d.ap().to_broadcast([P, D]))
                nc.sync.dma_start(out=QRB, in_=qrb_d.ap().rearrange("(c p) d -> p c d", p=P))

                def make_norm(qc, hp, PVS, rb):
                    qlo = qc * 1024

                    def norm():
                        for hb in range(2):
                            nc.gpsimd.tensor_tensor(
                                ATT[hb * 64 : (hb + 1) * 64, hp, qlo : qlo + 1024],
                                PVS[0:64, hb, :],
                                rb[:, hb, :],
                                OP.mult,
                            )

                    return norm

                def make_fc(qc, ch, sub, tag):
                    qlo = qc * 1024

                    def fc():
                        j, r = divmod(sub, 2)          # pair member, block
                        t = j * 4 + ch * 2 + r          # tokc within q-half
                        psfh = [
                            ps.tile([P, 512], F32, tag="sc", name=f"psf{h}", bufs=4)
                            for h in range(2)
                        ]
                        for half in range(2):
                            for dp in range(2):
                                nc.tensor.matmul(
                                    psfh[half],
                                    lhsT=ATT[
                                        :, 2 * dp : 2 * dp + 2, qlo + t * P : qlo + (t + 1) * P
                                    ],
                                    rhs=WOp[
                                        :, 2 * dp : 2 * dp + 2, half * 512 : (half + 1) * 512
                                    ],
                                    start=(dp == 0),
                                    stop=(dp == 1),
                                    perf_mode=DR,
                                )
                        fcs = late.tile([P, D], BF16, tag="fcs", bufs=3)
                        nc.scalar.mul(fcs[:, 0:512], psfh[0], 1.0 / 512.0)
                        nc.vector.tensor_scalar_mul(
                            out=fcs[:, 512:1024], in0=psfh[1], scalar1=1.0 / 512.0
                        )
                        nc.sync.dma_start(
                            out=cc_in[2 * qc + ch][j * 256 + r * P : j * 256 + (r + 1) * P, :],
                            in_=fcs,
                        )

                    return fc

                def make_rs(i):
                    def rs():
                        nc.gpsimd.collective_compute(
                            "ReduceScatter",
                            OP.add,
                            replica_groups=groups,
                            ins=[cc_in[i].ap().opt()],
                            outs=[cc_out[i].ap().opt()],
                        )

                    return rs

                def make_ln(qc, ch):
                    # interleaved LNs (qc==0) run while vector is exp-loaded:
                    # put the SBUF-only elementwise on gpsimd.  Tail LNs
                    # (qc==1) run after attention: vector is free and faster.
                    geng = nc.vector if qc == 1 else nc.gpsimd

                    def ln():
                        MV = late.tile([P, 2, 2], F32, tag="MV", bufs=2)
                        RST = late.tile([P, 2], F32, tag="RST", bufs=2)
                        xts = []
                        for r in range(2):
                            tc4 = ch * 2 + r
                            xc = late.tile([P, D], BF16, tag="xc", bufs=2)
                            nc.sync.dma_start(
                                out=xc, in_=cc_out[2 * qc + ch][r * P : (r + 1) * P, :]
                            )
                            xt = late.tile([P, D], F32, tag="xt", bufs=2)
                            xts.append(xt)
                            geng.tensor_tensor(xt, xc, QRB[:, qc * 4 + tc4, :], OP.add)
                            st = late.tile([P, 2, 6], F32, tag="st", bufs=2)
                            nc.vector.bn_stats(st[:, 0, :], xt[:, 0:512])
                            nc.vector.bn_stats(st[:, 1, :], xt[:, 512:1024])
                            nc.vector.bn_aggr(MV[:, r, :], st)
                            nc.vector.tensor_scalar_add(
                                out=RST[:, r : r + 1],
                                in0=MV[:, r, 1:2],
                                scalar1=LN_EPS,
                            )
                        nc.vector.reciprocal(RST, RST)
                        nc.scalar.activation(out=RST, in_=RST, func=AF.Sqrt)
                        for r in range(2):
                            tc4 = ch * 2 + r
                            xn = late.tile([P, D], F32, tag="xn", bufs=2)
                            geng.tensor_scalar(
                                out=xn,
                                in0=xts[r],
                                scalar1=MV[:, r, 0:1],
                                scalar2=RST[:, r : r + 1],
                                op0=OP.subtract,
                                op1=OP.mult,
                            )
                            geng.tensor_tensor(xn, xn, GAM, OP.mult)
                            geng.tensor_tensor(xn, xn, BET, OP.add)
                            nc.sync.dma_start(
                                out=out_d[qc * 512 + tc4 * P : qc * 512 + (tc4 + 1) * P, :],
                                in_=xn,
                            )

                    return ln

                # deferred-work queues: pend[k] emits k hp-windows from now
                pend = {1: [], 2: [], 3: [], 4: []}

                for qc in range(QC):
                    qlo = qc * 1024
                    for hp in range(4):
                        work = pend[1]
                        pend = {1: pend[2], 2: pend[3], 3: pend[4], 4: []}

                        EXPT = attp.tile([P, KC, 2048], FP8E5, tag="exp", bufs=1)
                        pvs = [
                            ps.tile([65, 1024], F32, tag="pv", name=f"pv{hb}")
                            for hb in range(2)
                        ]
                        for vkc in range(20):
                            if vkc < KC:
                                kc = vkc
                                klo = kc * P
                                for hb in range(2):
                                    plo = hb * 64
                                    for half in range(2):
                                        sc = ps.tile([P, 512], F32, tag="sc", name="sc", bufs=4)
                                        nc.tensor.matmul(
                                            sc,
                                            lhsT=KT[plo : plo + 64, hp, klo : klo + P],
                                            rhs=QT[
                                                plo : plo + 64,
                                                hp,
                                                qlo + half * 512 : qlo + (half + 1) * 512,
                                            ],
                                        )
                                        edst = EXPT[
                                            :, kc, hb * 1024 + half * 512 : hb * 1024 + (half + 1) * 512
                                        ]
                                        eng = exp_engine(kc * 4 + hb * 2 + half)
                                        if eng == "sca":
                                            nc.scalar.activation(
                                                out=edst, in_=sc, func=AF.Exp, scale=EXP_SCALE
                                            )
                                        else:
                                            nc.vector.tensor_scalar(
                                                out=edst.bitcast(U8),
                                                in0=sc,
                                                scalar1=C2,
                                                scalar2=0.0,
                                                op0=OP.add,
                                                op1=OP.max,
                                            )
                            # att@V trails the scores by 2.5 key-chunks
                            if vkc >= 5 and vkc % 2 == 1:
                                kp = (vkc - 5) // 2
                                for hb in range(2):
                                    for half in range(2):
                                        nc.tensor.matmul(
                                            pvs[hb][0:65, half * 512 : (half + 1) * 512],
                                            lhsT=VO[:, kp, :, 2 * hp + hb, 0:65],
                                            rhs=EXPT[
                                                :,
                                                2 * kp : 2 * kp + 2,
                                                hb * 1024 + half * 512 : hb * 1024 + (half + 1) * 512,
                                            ],
                                            start=(kp == 0),
                                            stop=(kp == 7),
                                            perf_mode=DR,
                                        )
                            # deferred items start at vkc 6 so their upstream
                            # DMA chains (reciprocal broadcast, cc_out) have
                            # landed — an in-order engine queue must never park.
                            # Exception: a leading gpsimd normalize may park at
                            # vkc 0 (nothing else queues on gpsimd).
                            if work and (vkc >= 6 or (vkc == 0 and getattr(work[0], "is_norm", False))):
                                work.pop(0)()
                        for w in work:
                            w()

                        # pv -> SBUF (frees PSUM), denominators -> reciprocal
                        PVS = attp.tile([65, 2, 1024], F32, tag="PVS", bufs=2)
                        rlo = qc * HL + 2 * hp
                        # denominator rows first (tiny) so the reciprocal
                        # chain launches before the bulk pv copies finish
                        nc.scalar.copy(PVS[64:65, 0, :], pvs[0][64:65, :])
                        nc.vector.tensor_copy(PVS[64:65, 1, :], pvs[1][64:65, :])
                        for hb in range(2):
                            nc.sync.dma_start(
                                out=sums_dram[rlo + hb : rlo + hb + 1, :],
                                in_=PVS[64:65, hb, :],
                            )
                        nc.scalar.copy(PVS[0:64, 0, :], pvs[0][0:64, :])
                        nc.vector.tensor_copy(PVS[0:64, 1, :], pvs[1][0:64, :])
                        SUI = attp.tile([2, 1024], F32, tag="SUI", bufs=1)
                        SUO = attp.tile([2, 1024], F32, tag="SUO", bufs=1)
                        nc.sync.dma_start(out=SUI, in_=sums_dram[rlo : rlo + 2, :])
                        nc.vector.reciprocal_approx_fast(SUO, SUI)
                        nc.sync.dma_start(out=rec_dram[rlo : rlo + 2, :], in_=SUO)
                        rb = attp.tile([64, 2, 1024], F32, tag="rb", bufs=2)
                        for hb in range(2):
                            nc.sync.dma_start(
                                out=rb[:, hb, :],
                                in_=rec_dram[rlo + hb : rlo + hb + 1, :].to_broadcast(
                                    [64, 1024]
                                ),
                            )
                        pend[1].append(make_norm(qc, hp, PVS, rb))

                    # queue this q-half's fc + RS + LN
                    if qc == 0:
                        for ch in range(2):
                            for sub in range(4):
                                pend[1].append(make_fc(qc, ch, sub, "psf"))
                            pend[1].append(make_rs(2 * qc + ch))
                            pend[3 + ch].append(make_ln(qc, ch))
                    else:
                        # final q-half: flush everything now (tail)
                        for w in pend[1]:
                            w()
                        for ch in range(2):
                            for sub in range(4):
                                make_fc(qc, ch, sub, "psf")()
                            make_rs(2 * qc + ch)()
                            make_ln(qc, ch)()

    nc.compile()
    return nc


def _f8(a):
    return np.ascontiguousarray(a).astype(ml_dtypes.float8_e4m3)


def kernel(q, k, v, Wq, bq, Wk, bk, Wv, bv, Wo, bo, gamma, beta, _trace=False):
    global _NC_CACHE, _LAST_RES
    q = np.asarray(q, np.float32)
    k = np.asarray(k, np.float32)
    v = np.asarray(v, np.float32)
    Wq, Wk, Wv, Wo = (np.asarray(w, np.float32) for w in (Wq, Wk, Wv, Wo))
    bq, bk, bv, bo = (np.asarray(x, np.float32) for x in (bq, bk, bv, bo))
    gamma = np.asarray(gamma, np.float32)
    beta = np.asarray(beta, np.float32)

    in_maps = []
    for c in range(8):
        b, g = divmod(c, 2)
        sl = slice(g * DH, (g + 1) * DH)
        qres = np.concatenate(
            [
                q[b, g * 512 : g * 512 + 512],
                q[b, 1024 + g * 512 : 1024 + g * 512 + 512],
            ]
        )
        in_maps.append(
            {
                "xqT": _f8(q[b].T),
                "xkT": _f8(k[b].T),
                "xvT": _f8(v[b].T),
                "wqT": _f8(Wq[sl, :].T * WS),
                "wkT": _f8(Wk[sl, :].T * WS),
                "wvT": _f8(Wv[sl, :].T * WS),
                "woT": _f8(Wo[:, sl].T * WS),
                "bq": (bq[sl] * A_QK).astype(np.float32),
                "bk": (bk[sl] * A_QK).astype(np.float32),
                "bv": (bv[sl] * WS).reshape(1, DH).astype(np.float32),
                "gamma": gamma.reshape(1, D).copy(),
                "beta": beta.reshape(1, D).copy(),
                "qrb": np.ascontiguousarray(qres + bo[None, :]).astype(
                    ml_dtypes.bfloat16
                ),
            }
        )

    if _NC_CACHE is None:
        _NC_CACHE = build_nc()
    nc = _NC_CACHE

    kw = {}
    if _trace:
        import tempfile

        kw = dict(trace=True, tmpdir=tempfile.mkdtemp(prefix="mha_trace_"))
    res = run_bass_kernel_spmd(nc, in_maps, list(range(8)), **kw)
    _LAST_RES = res

    out = np.empty((B, S, D), np.float32)
    for c in range(8):
        b, g = divmod(c, 2)
        r = res.results[c]["out"]
        out[b, g * 512 : g * 512 + 512] = r[0:512]
        out[b, 1024 + g * 512 : 1024 + g * 512 + 512] = r[512:1024]

    if _trace:
        kernel._last = res
    return out
